# revision 1
# baseline (speedup 1.0000x reference)
"""Trainium2 Bass kernel for nn_DFTQNN: 8 sequential gates psi <- expm(-i*theta_g*G_g) @ psi,
output |psi|^2.

Algorithm: instead of materializing expm (matrix-matrix, ~20 GFLOP/gate), apply each gate's
exponential directly to the statevector with a Chebyshev expansion:

    exp(-i t G) v = sum_k c_k T_k(G/lam) v,   c_k = mu_k (-i sgn)^k J_k(|t| lam)

where lam >= ||G||_2 (host-side power iteration), J_k = Bessel. The T_k(G/lam) v iterates
satisfy w_{k+1} = (2/lam) G w_k - w_{k-1} -- a chain of ~6-10 1024x1024 matvecs per gate
(~60 total), ~1000x fewer flops than the reference expm path.

Device mapping (per NeuronCore, fully replicated across the 8 cores -- the chain is
serial and cross-core collectives cost ~5us/call, far more than they could save):
  - statevector w kept in "chunk layout" [128 part, 16 free]: partition r, col 2q+s
    holds component s (re/im) of element m = 128q + r.
  - per term: y[m] = sum_n G[n, m] w[n] (G symmetric) via 32 accumulating fp32r matmuls:
    8 contraction chunks x 4 column-tiled groups (tile_position=(0,32j), N=256 streams
    run concurrently on separate XBUSes; fp32r streams 1 row/cycle at N>=256).
  - y lands free-major in PSUM [2-of-32 part rows, 256]; ScalarE copies it to SBUF with
    the 2/lam Chebyshev scale folded into the activation scale; 8 PE transposes
    [2,128] -> [128,2] return it to chunk layout; VectorE applies the three-term
    recurrence and the c_k accumulation (pure-real/pure-imag alternation = strided
    column APs, no cross-partition work).
"""

import os
import numpy as np

DIM = 1024
P = 128
NCH = DIM // P        # 8 contraction chunks
NGRP = 4              # column-tile groups
GW = DIM // NGRP      # 256 stream columns per group
NGATE = 8
CHEB_TOL = 1e-4
KMAX = 48


# ---------------------------------------------------------------- host math
def _bessel_j(kmax, z, npts=2048):
    t = np.linspace(0.0, np.pi, npts + 1)
    k = np.arange(kmax + 1)[:, None]
    f = np.cos(k * t[None, :] - z * np.sin(t)[None, :])
    return np.trapezoid(f, t, axis=1) / np.pi


def _lam_max(G, iters=60, seed=0):
    rng = np.random.default_rng(seed)
    v = rng.standard_normal(G.shape[0])
    v /= np.linalg.norm(v)
    lam = 1.0
    for _ in range(iters):
        w = G @ v
        lam = np.linalg.norm(w)
        v = w / lam
    return float(lam)


def _cheb_coeffs(theta, lam, tol=CHEB_TOL, kmax=KMAX):
    z = theta * lam
    J = _bessel_j(kmax, abs(z))
    sgn = 1.0 if z >= 0 else -1.0
    c = np.zeros(kmax + 1, dtype=np.complex128)
    c[0] = J[0]
    for k in range(1, kmax + 1):
        c[k] = 2.0 * ((-1j * sgn) ** k) * J[k]
    mags = np.abs(c)
    K = 0
    acc = 0.0
    for k in range(kmax, 0, -1):
        acc += mags[k]
        if acc > tol:
            K = min(k + 1, kmax)
            break
    return c, K


# ---------------------------------------------------------------- device program
def build_program(lams, coeffs, Ks):
    import concourse.bass as bass
    import concourse.mybir as mybir
    import concourse.tile as tile
    from concourse import bacc
    from contextlib import ExitStack

    f32 = mybir.dt.float32
    f32r = mybir.dt.float32r
    Copy = mybir.ActivationFunctionType.Copy
    Alu = mybir.AluOpType

    nc = bacc.Bacc("TRN2", target_bir_lowering=False, debug=False, num_devices=8)

    feat_d = nc.dram_tensor("feature", [DIM], f32, kind="ExternalInput")
    gens_d = nc.dram_tensor("gens", [NGATE, DIM, DIM], f32, kind="ExternalInput")
    out_d = nc.dram_tensor("out", [DIM], f32, kind="ExternalOutput")

    with tile.TileContext(nc) as tc, ExitStack() as ctx:
        const = ctx.enter_context(tc.tile_pool(name="const", bufs=1))
        gpool = ctx.enter_context(tc.tile_pool(name="gpool", bufs=3))
        spool = ctx.enter_context(tc.tile_pool(name="spool", bufs=2))
        wpool = ctx.enter_context(tc.tile_pool(name="wpool", bufs=3))
        apool = ctx.enter_context(tc.tile_pool(name="apool", bufs=2))
        pspool = ctx.enter_context(tc.tile_pool(name="ps", bufs=1, space="PSUM"))
        psmall = ctx.enter_context(tc.tile_pool(name="pssm", bufs=1, space="PSUM"))

        # ---- constants
        id_t = const.tile([2, 2], f32)
        nc.gpsimd.memset(id_t[:], 0.0)
        nc.gpsimd.affine_select(
            out=id_t[:],
            in_=id_t[:],
            compare_op=Alu.not_equal,
            fill=1.0,
            base=0,
            pattern=[[-1, 2]],
            channel_multiplier=1,
        )
        ones_col = const.tile([P, 1], f32)
        nc.vector.memset(ones_col[:], 1.0)
        ones_row = const.tile([1, P], f32)
        nc.vector.memset(ones_row[:], 1.0)

        # ---- feature embedding + 1/||f||^2
        f_emb = apool.tile([P, 2 * NCH], f32, tag="acc")
        nc.vector.memset(f_emb[:], 0.0)
        f_even = f_emb[:].rearrange("p (q s) -> p q s", s=2)[:, :, 0]
        # feature[m], m = 128 q + r  ->  partition r, col q
        nc.sync.dma_start(out=f_even, in_=feat_d.ap().rearrange("(q r) -> r q", r=P))

        sq = const.tile([P, NCH], f32)
        nc.vector.tensor_tensor(out=sq[:], in0=f_even, in1=f_even, op=Alu.mult)
        rsum = const.tile([P, 1], f32)
        nc.vector.tensor_reduce(
            out=rsum[:], in_=sq[:], axis=mybir.AxisListType.X, op=Alu.add
        )
        n2_ps = psmall.tile([1, 1], f32, tag="n2")
        nc.tensor.matmul(out=n2_ps[:], lhsT=ones_col[:], rhs=rsum[:], start=True, stop=True)
        inv_sb = const.tile([1, 1], f32)
        nc.vector.reciprocal(out=inv_sb[:], in_=n2_ps[:])
        invb_ps = psmall.tile([P, 1], f32, tag="invb")
        nc.tensor.matmul(out=invb_ps[:], lhsT=ones_row[:], rhs=inv_sb[:], start=True, stop=True)
        invb = const.tile([P, 1], f32)
        nc.vector.tensor_copy(invb[:], invb_ps[:])

        # ---- persistent PSUM tiles (fully written each term; no memset needed)
        Y = [pspool.tile([2, 512], f32, tag=f"Y{i}", name=f"Y{i}") for i in range(2)]
        Tt = pspool.tile([P, 2 * NCH], f32, tag="T", name="Tps")

        # ---- gate chain
        w_cur = f_emb  # w_0 of gate 0 (acc-pool tile)
        term_idx = 0
        for g in range(NGATE):
            lam = lams[g]
            c = coeffs[g]
            K = Ks[g]

            # raw G chunks: gt[c][k, m] = G[128 c + k, m]; DMA fp32 staging then
            # cast to the fp32r encoding on the Scalar engine
            gts = []
            for ch in range(NCH):
                gs = gpool.tile([P, DIM], f32, tag=f"Gs{ch}", name=f"Gs{g}_{ch}")
                nc.sync.dma_start(out=gs[:], in_=gens_d.ap()[g, 128 * ch : 128 * (ch + 1), :])
                gt = gpool.tile([P, DIM], f32r, tag=f"G{ch}", name=f"G{g}_{ch}")
                nc.scalar.activation(out=gt[:], in_=gs[:], func=Copy)
                gts.append(gt)

            acc = apool.tile([P, 2 * NCH], f32, tag="acc", name=f"acc{g}")
            # acc = c0 * w0
            nc.vector.tensor_scalar_mul(acc[:], w_cur[:], float(c[0].real))

            w_prev = None
            for k in range(1, K + 1):
                Sa = spool.tile([2, 512], f32, tag="Sa", name=f"Sa{term_idx}")
                Sb = spool.tile([2, 512], f32, tag="Sb", name=f"Sb{term_idx}")
                w_r = wpool.tile([P, 2 * NCH], f32r, tag="wr", name=f"wr{term_idx}")
                nc.vector.tensor_copy(w_r[:, 0:8], w_cur[:, 0:8])
                nc.vector.tensor_copy(w_r[:, 8:16], w_cur[:, 8:16])
                term_idx += 1

                # y = G^T w  (16 accumulating fp32r matmuls, N=512, two PSUM banks)
                # half-major order: Y[0]'s copy/transposes overlap Y[1]'s matmuls
                scale = (1.0 / lam) if k == 1 else (2.0 / lam)
                w_next = wpool.tile([P, 2 * NCH], f32, tag="w", name=f"w{g}_{k}")
                for half in range(2):
                    for ch in range(NCH):
                        nc.tensor.matmul(
                            out=Y[half][:],
                            lhsT=w_r[:, 2 * ch : 2 * ch + 2],
                            rhs=gts[ch][:, 512 * half : 512 * (half + 1)].bitcast(f32r),
                            start=(ch == 0),
                            stop=(ch == NCH - 1),
                            tile_position=(0, 0),
                        )
                    if half == 0:
                        # S = (2/lam) y  (k==1: 1/lam); DVE while PE streams half 1
                        nc.vector.tensor_scalar_mul(Sa[:], Y[0][:], float(scale))
                    else:
                        # ACT, split in two so transposes q=4,5 start sooner
                        nc.scalar.activation(out=Sb[:, 0:256], in_=Y[1][:, 0:256], func=Copy, scale=float(scale))
                        nc.scalar.activation(out=Sb[:, 256:512], in_=Y[1][:, 256:512], func=Copy, scale=float(scale))

                # 8 transposes [2,128] -> [128,2] back to chunk layout, then the
                # recurrence per half so next term's first matmuls start early
                for q in range(2 * NCH // 2):
                    Sx = Sa if q < 4 else Sb
                    h = q % 4
                    nc.tensor.matmul(
                        Tt[:, 2 * q : 2 * q + 2],
                        Sx[:, P * h : P * (h + 1)],
                        id_t[:],
                        is_transpose=True,
                        start=(q == 0),
                        stop=(q == NCH - 1),
                        tile_position=(0, 0),
                    )
                    if q == 3 or q == NCH - 1:
                        cols = slice(0, 8) if q == 3 else slice(8, 16)
                        if k == 1:
                            nc.vector.tensor_copy(w_next[:, cols], Tt[:, cols])
                        else:
                            nc.vector.tensor_tensor(
                                out=w_next[:, cols], in0=Tt[:, cols],
                                in1=w_prev[:, cols], op=Alu.subtract,
                            )

                # acc += c_k * sigma_k(w)  (fused multiply-add on DVE)
                ck = c[k]
                if k % 2 == 0:
                    nc.vector.scalar_tensor_tensor(
                        out=acc[:], in0=w_next[:], scalar=float(ck.real),
                        in1=acc[:], op0=Alu.mult, op1=Alu.add,
                    )
                else:
                    wv = w_next[:].rearrange("p (q s) -> p q s", s=2)
                    av = acc[:].rearrange("p (q s) -> p q s", s=2)
                    b = float(ck.imag)
                    nc.vector.scalar_tensor_tensor(
                        out=av[:, :, 0], in0=wv[:, :, 1], scalar=-b,
                        in1=av[:, :, 0], op0=Alu.mult, op1=Alu.add,
                    )
                    nc.vector.scalar_tensor_tensor(
                        out=av[:, :, 1], in0=wv[:, :, 0], scalar=b,
                        in1=av[:, :, 1], op0=Alu.mult, op1=Alu.add,
                    )

                w_prev, w_cur = w_cur, w_next

            w_cur = acc  # unnormalized psi after gate g

        # ---- output: |psi|^2 / ||f||^2
        sq2 = const.tile([P, 2 * NCH], f32)
        nc.vector.tensor_tensor(out=sq2[:], in0=w_cur[:], in1=w_cur[:], op=Alu.mult)
        sv = sq2[:].rearrange("p (q s) -> p q s", s=2)
        prob = const.tile([P, NCH], f32)
        nc.vector.tensor_tensor(out=prob[:], in0=sv[:, :, 0], in1=sv[:, :, 1], op=Alu.add)
        nc.vector.tensor_scalar_mul(prob[:], prob[:], invb[:])
        nc.sync.dma_start(out=out_d.ap().rearrange("(q r) -> r q", r=P), in_=prob[:])

    nc.compile()
    return nc


# ---------------------------------------------------------------- entry point
_CACHE = {}


def _prep(theta, gens):
    lams = [_lam_max(gens[g].astype(np.float64)) * 1.03 for g in range(NGATE)]
    coeffs, Ks = [], []
    for g in range(NGATE):
        c, K = _cheb_coeffs(float(theta[g, 0]), lams[g])
        coeffs.append(c)
        Ks.append(max(K, 1))
    return lams, coeffs, Ks


def kernel(feature, theta, gens):
    from concourse.bass_utils import run_bass_kernel_spmd

    feature = np.ascontiguousarray(feature, dtype=np.float32)
    theta = np.ascontiguousarray(theta, dtype=np.float32)
    gens = np.ascontiguousarray(gens, dtype=np.float32)

    lams, coeffs, Ks = _prep(theta, gens)
    key = (theta.tobytes(), tuple(np.round(lams, 9)), tuple(Ks))
    if key not in _CACHE:
        _CACHE[key] = build_program(lams, coeffs, Ks)
    nc = _CACHE[key]

    in_map = {"feature": feature, "gens": gens}
    res = run_bass_kernel_spmd(
        nc,
        [dict(in_map) for _ in range(8)],
        core_ids=list(range(8)),
        trace=False,
    )
    return np.asarray(res.results[0]["out"], dtype=np.float32)


if __name__ == "__main__":
    d = np.load("/root/problem/ref_cache.npz")
    out = kernel(d["feature"], d["theta"], d["gens"])
    exp = d["expected"]
    rel = np.linalg.norm(out - exp) / np.linalg.norm(exp)
    print("l2 rel err:", rel)
    print("max abs err:", np.abs(out - exp).max())



# revision 5
# speedup vs baseline: 1.5542x; 1.5542x over previous
"""Trainium2 Bass kernel for nn_DFTQNN: 8 sequential gates psi <- expm(-i*theta_g*G_g) @ psi,
output |psi|^2.

Chebyshev expansion applied directly to the statevector (no expm matrix):
    exp(-i t G) v = sum_k c_k T_k(G/lam) v,   c_k = mu_k (-i sgn)^k J_k(|t| lam)
with the T_k recurrence evaluated in the rescaled basis v_k = (lam/2)^k T_k(G/lam) v:
    v_1 = 0.5 G v_0,   v_k = G v_{k-1} - (lam/2)^2 v_{k-2},   chat_k = c_k (2/lam)^k
so the raw G streams through the PE with no prescale pass. All matvec operands are
bf16 (required for PE column tiling; validated 4.5e-3 rel err vs the 2e-2 gate), which
also halves the G HBM traffic (16 MB total).

Device mapping (per NeuronCore, replicated across 8 cores -- the chain is serial and
collectives cost more than they save):
  - statevector in chunk layout [128 part, 16 free]: partition r, col 2q+s holds
    component s (re/im) of element m = 128q + r.
  - matvec y = G v via 32 accumulating bf16 matmuls: 4 column-tiled groups
    (tile_position=(0,32j), N=256) x 8 contraction chunks. The 4 groups stream
    CONCURRENTLY through disjoint 32-col strips of the PE array (separate XBUSes),
    ~4x the single-stream rate; issue round-robin chunk-major so each group's
    accumulation chain stays back-to-back.
  - Y lands at PSUM partitions {32j, 32j+1} x 256; group copies (2 on DVE, 2 on ACT)
    stage it to SBUF; 8 PE transposes [2,128] -> [128,2] restore chunk layout into a
    second PSUM bank; DVE applies the three-term recurrence (fused subtract) and the
    chat_k accumulation. PSUM tags ping-pong (bufs=2) so term k+1's matmuls overlap
    term k's tail.
"""

import numpy as np

DIM = 1024
P = 128
NCH = DIM // P        # 8 contraction chunks
NGRP = 4              # column-tiled groups
GW = DIM // NGRP      # 256 stream columns per group
NGATE = 8
CHEB_TOL = 1e-2
KMAX = 48


# ---------------------------------------------------------------- host math
def _bessel_j(kmax, z, npts=2048):
    t = np.linspace(0.0, np.pi, npts + 1)
    k = np.arange(kmax + 1)[:, None]
    f = np.cos(k * t[None, :] - z * np.sin(t)[None, :])
    return np.trapezoid(f, t, axis=1) / np.pi


def _lam_max(G, iters=60, seed=0):
    rng = np.random.default_rng(seed)
    v = rng.standard_normal(G.shape[0])
    v /= np.linalg.norm(v)
    lam = 1.0
    for _ in range(iters):
        w = G @ v
        lam = np.linalg.norm(w)
        v = w / lam
    return float(lam)


def _cheb_coeffs(theta, lam, tol=CHEB_TOL, kmax=KMAX):
    z = theta * lam
    J = _bessel_j(kmax, abs(z))
    sgn = 1.0 if z >= 0 else -1.0
    c = np.zeros(kmax + 1, dtype=np.complex128)
    c[0] = J[0]
    for k in range(1, kmax + 1):
        c[k] = 2.0 * ((-1j * sgn) ** k) * J[k]
    mags = np.abs(c)
    K = 0
    acc = 0.0
    for k in range(kmax, 0, -1):
        acc += mags[k]
        if acc > tol:
            K = min(k + 1, kmax)
            break
    return c, max(K, 1)


# ---------------------------------------------------------------- device program
def build_program(lams, coeffs, Ks):
    import concourse.bass as bass
    import concourse.mybir as mybir
    import concourse.tile as tile
    from concourse import bacc
    from contextlib import ExitStack

    f32 = mybir.dt.float32
    f32r = mybir.dt.float32r
    bf16 = mybir.dt.bfloat16
    Copy = mybir.ActivationFunctionType.Copy
    Alu = mybir.AluOpType

    nc = bacc.Bacc("TRN2", target_bir_lowering=False, debug=False, num_devices=8)

    feat_d = nc.dram_tensor("feature", [DIM], f32, kind="ExternalInput")
    gens_d = nc.dram_tensor("gens", [NGATE, DIM, DIM], bf16, kind="ExternalInput")
    out_d = nc.dram_tensor("out", [DIM], f32, kind="ExternalOutput")

    with tile.TileContext(nc) as tc, ExitStack() as ctx:
        const = ctx.enter_context(tc.tile_pool(name="const", bufs=1))
        gpool = ctx.enter_context(tc.tile_pool(name="gpool", bufs=3))
        fmpool = ctx.enter_context(tc.tile_pool(name="fmpool", bufs=2))
        wpool = ctx.enter_context(tc.tile_pool(name="wpool", bufs=3))
        apool = ctx.enter_context(tc.tile_pool(name="apool", bufs=2))
        pspool = ctx.enter_context(tc.tile_pool(name="ps", bufs=2, space="PSUM"))
        psmall = ctx.enter_context(tc.tile_pool(name="pssm", bufs=1, space="PSUM"))

        # ---- constants: per-group 2x2 identities at partitions {32j, 32j+1}
        idall = const.tile([P, 2], f32)
        nc.gpsimd.memset(idall[:], 0.0)
        for j in range(NGRP):
            nc.gpsimd.affine_select(
                out=idall[32 * j : 32 * j + 2, :],
                in_=idall[32 * j : 32 * j + 2, :],
                compare_op=Alu.not_equal,
                fill=1.0,
                base=0,
                pattern=[[-1, 2]],
                channel_multiplier=1,
            )
        ones_col = const.tile([P, 1], f32)
        nc.vector.memset(ones_col[:], 1.0)
        ones_row = const.tile([1, P], f32)
        nc.vector.memset(ones_row[:], 1.0)

        # ---- feature embedding + 1/||f||^2
        f_emb = apool.tile([P, 2 * NCH], f32, tag="acc")
        nc.vector.memset(f_emb[:], 0.0)
        f_even = f_emb[:].rearrange("p (q s) -> p q s", s=2)[:, :, 0]
        # feature[m], m = 128 q + r  ->  partition r, col q
        nc.sync.dma_start(out=f_even, in_=feat_d.ap().rearrange("(q r) -> r q", r=P))

        sq = const.tile([P, NCH], f32)
        nc.vector.tensor_tensor(out=sq[:], in0=f_even, in1=f_even, op=Alu.mult)
        rsum = const.tile([P, 1], f32)
        nc.vector.tensor_reduce(
            out=rsum[:], in_=sq[:], axis=mybir.AxisListType.X, op=Alu.add
        )
        n2_ps = psmall.tile([1, 1], f32, tag="n2")
        nc.tensor.matmul(out=n2_ps[:], lhsT=ones_col[:], rhs=rsum[:], start=True, stop=True)
        inv_sb = const.tile([1, 1], f32)
        nc.vector.reciprocal(out=inv_sb[:], in_=n2_ps[:])
        invb_ps = psmall.tile([P, 1], f32, tag="invb")
        nc.tensor.matmul(out=invb_ps[:], lhsT=ones_row[:], rhs=inv_sb[:], start=True, stop=True)
        invb = const.tile([P, 1], f32)
        nc.vector.tensor_copy(invb[:], invb_ps[:])

        # ---- gate chain
        w_cur = f_emb  # v_0 of gate 0 (chunk layout, acc-pool tile)
        term_idx = 0
        for g in range(NGATE):
            lam = lams[g]
            c = coeffs[g]
            K = Ks[g]
            beta2 = float((lam / 2.0) ** 2)

            # raw G chunks, DMA'd straight from DRAM; bitcast to fp32r at use
            gts = []
            for ch in range(NCH):
                gt = gpool.tile([P, DIM], bf16, tag=f"G{ch}", name=f"G{g}_{ch}")
                nc.sync.dma_start(out=gt[:], in_=gens_d.ap()[g, P * ch : P * (ch + 1), :])
                gts.append(gt)

            # v0 in bf16 form for the k=1 matvec lhsT (DVE cast rounds)
            v0r = wpool.tile([P, 2 * NCH], bf16, tag="w", name=f"v0r{g}")
            nc.vector.tensor_copy(v0r[:], w_cur[:])
            acc = apool.tile([P, 2 * NCH], f32, tag="acc", name=f"acc{g}")
            # acc = c0 * v0
            nc.vector.tensor_scalar_mul(acc[:], w_cur[:], float(c[0].real))
            w_cur = v0r

            w_prev = None  # v_{k-2}
            for k in range(1, K + 1):
                term_idx += 1
                # --- Y[32j:32j+2, :] = sum_ch G_ch[:, grp j]^T v_{k-1}  (4 groups concurrent)
                Y = pspool.tile([P, GW], f32, tag="Y", name=f"Y{term_idx}")
                for ch in range(NCH):
                    lhs = w_cur[:, 2 * ch : 2 * ch + 2]
                    for j in range(NGRP):
                        nc.tensor.matmul(
                            out=Y[32 * j : 32 * j + 2, :],
                            lhsT=lhs,
                            rhs=gts[ch][:, GW * j : GW * (j + 1)],
                            start=(ch == 0),
                            stop=(ch == NCH - 1),
                            tile_position=(0, 32 * j),
                        )

                # --- stage PSUM -> SBUF per group (k==1 folds the 0.5 of v1 = 0.5 G v0)
                fm = fmpool.tile([P, GW], f32, tag="fm", name=f"fm{term_idx}")
                s05 = 0.5 if k == 1 else 1.0
                # groups 0,2 on DVE; 1,3 on ACT (parallel engines)
                if k == 1:
                    nc.vector.tensor_scalar_mul(fm[0:2, :], Y[0:2, :], s05)
                    nc.vector.tensor_scalar_mul(fm[64:66, :], Y[64:66, :], s05)
                else:
                    nc.vector.tensor_copy(fm[0:2, :], Y[0:2, :])
                    nc.vector.tensor_copy(fm[64:66, :], Y[64:66, :])
                nc.scalar.activation(out=fm[32:34, :], in_=Y[32:34, :], func=Copy, scale=s05)
                nc.scalar.activation(out=fm[96:98, :], in_=Y[96:98, :], func=Copy, scale=s05)

                # --- 8 transposes [2,128]@32j -> [128,2] chunk layout; recurrence per half
                Tt = pspool.tile([P, 2 * NCH], f32, tag="T", name=f"T{term_idx}")
                w_next = wpool.tile([P, 2 * NCH], bf16, tag="w", name=f"w{g}_{k}")
                for q in range(2 * NCH // 2):
                    j, h = q // 2, q % 2
                    nc.tensor.matmul(
                        Tt[:, 2 * q : 2 * q + 2],
                        fm[32 * j : 32 * j + 2, P * h : P * (h + 1)],
                        idall[32 * j : 32 * j + 2, :],
                        is_transpose=True,
                        start=(q == 0),
                        stop=(q == NCH - 1),
                        tile_position=(32 * j, 0),
                    )
                    if q == 3 or q == NCH - 1:
                        cols = slice(0, 8) if q == 3 else slice(8, 16)
                        if k == 1:
                            nc.vector.tensor_copy(w_next[:, cols], Tt[:, cols])
                        else:
                            # v_k = Tt - beta2 * v_{k-2}
                            nc.vector.scalar_tensor_tensor(
                                out=w_next[:, cols], in0=w_prev[:, cols],
                                scalar=-beta2, in1=Tt[:, cols],
                                op0=Alu.mult, op1=Alu.add,
                            )

                # --- acc += chat_k * sigma_k(v_k)
                ck = c[k] * (2.0 / lam) ** k
                if k % 2 == 0:
                    nc.vector.scalar_tensor_tensor(
                        out=acc[:], in0=w_next[:], scalar=float(ck.real),
                        in1=acc[:], op0=Alu.mult, op1=Alu.add,
                    )
                else:
                    wv = w_next[:].rearrange("p (q s) -> p q s", s=2)
                    av = acc[:].rearrange("p (q s) -> p q s", s=2)
                    b = float(ck.imag)
                    nc.vector.scalar_tensor_tensor(
                        out=av[:, :, 0], in0=wv[:, :, 1], scalar=-b,
                        in1=av[:, :, 0], op0=Alu.mult, op1=Alu.add,
                    )
                    nc.vector.scalar_tensor_tensor(
                        out=av[:, :, 1], in0=wv[:, :, 0], scalar=b,
                        in1=av[:, :, 1], op0=Alu.mult, op1=Alu.add,
                    )

                w_prev, w_cur = w_cur, w_next

            w_cur = acc  # unnormalized psi after gate g

        # ---- output: |psi|^2 / ||f||^2
        sq2 = const.tile([P, 2 * NCH], f32)
        nc.vector.tensor_tensor(out=sq2[:], in0=w_cur[:], in1=w_cur[:], op=Alu.mult)
        sv = sq2[:].rearrange("p (q s) -> p q s", s=2)
        prob = const.tile([P, NCH], f32)
        nc.vector.tensor_tensor(out=prob[:], in0=sv[:, :, 0], in1=sv[:, :, 1], op=Alu.add)
        nc.vector.tensor_scalar_mul(prob[:], prob[:], invb[:])
        nc.sync.dma_start(out=out_d.ap().rearrange("(q r) -> r q", r=P), in_=prob[:])

    nc.compile()
    return nc


# ---------------------------------------------------------------- entry point
_CACHE = {}


def _prep(theta, gens):
    lams = [_lam_max(gens[g].astype(np.float64)) * 1.03 for g in range(NGATE)]
    coeffs, Ks = [], []
    for g in range(NGATE):
        c, K = _cheb_coeffs(float(theta[g, 0]), lams[g])
        coeffs.append(c)
        Ks.append(max(K, 1))
    return lams, coeffs, Ks


def kernel(feature, theta, gens):
    from concourse.bass_utils import run_bass_kernel_spmd

    import ml_dtypes

    feature = np.ascontiguousarray(feature, dtype=np.float32)
    theta = np.ascontiguousarray(theta, dtype=np.float32)
    gens = np.ascontiguousarray(gens, dtype=np.float32)
    gens_bf = np.ascontiguousarray(gens.astype(ml_dtypes.bfloat16))

    lams, coeffs, Ks = _prep(theta, gens)
    key = (theta.tobytes(), tuple(np.round(lams, 9)), tuple(Ks))
    if key not in _CACHE:
        _CACHE[key] = build_program(lams, coeffs, Ks)
    nc = _CACHE[key]

    in_map = {"feature": feature, "gens": gens_bf}
    res = run_bass_kernel_spmd(
        nc,
        [dict(in_map) for _ in range(8)],
        core_ids=list(range(8)),
        trace=False,
    )
    return np.asarray(res.results[0]["out"], dtype=np.float32)


if __name__ == "__main__":
    d = np.load("/root/problem/ref_cache.npz")
    out = kernel(d["feature"], d["theta"], d["gens"])
    exp = d["expected"]
    rel = np.linalg.norm(out - exp) / np.linalg.norm(exp)
    print("l2 rel err:", rel)
    print("max abs err:", np.abs(out - exp).max())


# revision 6
# speedup vs baseline: 1.6763x; 1.0785x over previous
"""Trainium2 Bass kernel for nn_DFTQNN: 8 sequential gates psi <- expm(-i*theta_g*G_g) @ psi,
output |psi|^2.

Chebyshev expansion applied directly to the statevector (no expm matrix):
    exp(-i t G) v = sum_k c_k T_k(G/lam) v,   c_k = mu_k (-i sgn)^k J_k(|t| lam)
with the T_k recurrence evaluated in the rescaled basis v_k = (lam/2)^k T_k(G/lam) v:
    v_1 = 0.5 G v_0,   v_k = G v_{k-1} - (lam/2)^2 v_{k-2},   chat_k = c_k (2/lam)^k
so the raw G streams through the PE with no prescale pass. All matvec operands are
bf16 (required for PE column tiling; validated 4.5e-3 rel err vs the 2e-2 gate), which
also halves the G HBM traffic (16 MB total).

Device mapping (per NeuronCore, replicated across 8 cores -- the chain is serial and
collectives cost more than they save):
  - statevector in chunk layout [128 part, 16 free]: partition r, col 2q+s holds
    component s (re/im) of element m = 128q + r.
  - matvec y = G v via 32 accumulating bf16 matmuls: 4 column-tiled groups
    (tile_position=(0,32j), N=256) x 8 contraction chunks. The 4 groups stream
    CONCURRENTLY through disjoint 32-col strips of the PE array (separate XBUSes),
    ~4x the single-stream rate; issue round-robin chunk-major so each group's
    accumulation chain stays back-to-back.
  - Y lands at PSUM partitions {32j, 32j+1} x 256; group copies (2 on DVE, 2 on ACT)
    stage it to SBUF; 8 PE transposes [2,128] -> [128,2] restore chunk layout into a
    second PSUM bank; DVE applies the three-term recurrence (fused subtract) and the
    chat_k accumulation. PSUM tags ping-pong (bufs=2) so term k+1's matmuls overlap
    term k's tail.
"""

import numpy as np

DIM = 1024
P = 128
NCH = DIM // P        # 8 contraction chunks
NGRP = 4              # column-tiled groups
GW = DIM // NGRP      # 256 stream columns per group
NGATE = 8
CHEB_TOL = 1e-2
KMAX = 48


# ---------------------------------------------------------------- host math
def _bessel_j(kmax, z, npts=2048):
    t = np.linspace(0.0, np.pi, npts + 1)
    k = np.arange(kmax + 1)[:, None]
    f = np.cos(k * t[None, :] - z * np.sin(t)[None, :])
    return np.trapezoid(f, t, axis=1) / np.pi


def _lam_max(G, iters=60, seed=0):
    rng = np.random.default_rng(seed)
    v = rng.standard_normal(G.shape[0])
    v /= np.linalg.norm(v)
    lam = 1.0
    for _ in range(iters):
        w = G @ v
        lam = np.linalg.norm(w)
        v = w / lam
    return float(lam)


def _cheb_coeffs(theta, lam, tol=CHEB_TOL, kmax=KMAX):
    z = theta * lam
    J = _bessel_j(kmax, abs(z))
    sgn = 1.0 if z >= 0 else -1.0
    c = np.zeros(kmax + 1, dtype=np.complex128)
    c[0] = J[0]
    for k in range(1, kmax + 1):
        c[k] = 2.0 * ((-1j * sgn) ** k) * J[k]
    mags = np.abs(c)
    K = 0
    acc = 0.0
    for k in range(kmax, 0, -1):
        acc += mags[k]
        if acc > tol:
            K = min(k + 1, kmax)
            break
    return c, max(K, 1)


# ---------------------------------------------------------------- device program
def build_program(lams, coeffs, Ks):
    import concourse.bass as bass
    import concourse.mybir as mybir
    import concourse.tile as tile
    from concourse import bacc
    from contextlib import ExitStack

    f32 = mybir.dt.float32
    f32r = mybir.dt.float32r
    bf16 = mybir.dt.bfloat16
    Copy = mybir.ActivationFunctionType.Copy
    Alu = mybir.AluOpType

    nc = bacc.Bacc("TRN2", target_bir_lowering=False, debug=False, num_devices=8)

    feat_d = nc.dram_tensor("feature", [DIM], f32, kind="ExternalInput")
    gens_d = nc.dram_tensor("gens", [NGATE, DIM, DIM], bf16, kind="ExternalInput")
    out_d = nc.dram_tensor("out", [DIM], f32, kind="ExternalOutput")

    with tile.TileContext(nc) as tc, ExitStack() as ctx:
        const = ctx.enter_context(tc.tile_pool(name="const", bufs=1))
        gpool = ctx.enter_context(tc.tile_pool(name="gpool", bufs=3))
        fmpool = ctx.enter_context(tc.tile_pool(name="fmpool", bufs=2))
        wpool = ctx.enter_context(tc.tile_pool(name="wpool", bufs=3))
        apool = ctx.enter_context(tc.tile_pool(name="apool", bufs=2))
        pspool = ctx.enter_context(tc.tile_pool(name="ps", bufs=2, space="PSUM"))
        psmall = ctx.enter_context(tc.tile_pool(name="pssm", bufs=1, space="PSUM"))

        # ---- constants: per-group 2x2 identities at partitions {32j, 32j+1}
        idall = const.tile([P, 2], f32)
        nc.gpsimd.memset(idall[:], 0.0)
        for j in range(NGRP):
            nc.gpsimd.affine_select(
                out=idall[32 * j : 32 * j + 2, :],
                in_=idall[32 * j : 32 * j + 2, :],
                compare_op=Alu.not_equal,
                fill=1.0,
                base=0,
                pattern=[[-1, 2]],
                channel_multiplier=1,
            )
        ones_col = const.tile([P, 1], f32)
        nc.vector.memset(ones_col[:], 1.0)
        ones_row = const.tile([1, P], f32)
        nc.vector.memset(ones_row[:], 1.0)

        # ---- feature embedding + 1/||f||^2
        f_emb = apool.tile([P, 2 * NCH], f32, tag="acc")
        nc.vector.memset(f_emb[:], 0.0)
        f_even = f_emb[:].rearrange("p (q s) -> p q s", s=2)[:, :, 0]
        # feature[m], m = 128 q + r  ->  partition r, col q
        nc.sync.dma_start(out=f_even, in_=feat_d.ap().rearrange("(q r) -> r q", r=P))

        sq = const.tile([P, NCH], f32)
        nc.vector.tensor_tensor(out=sq[:], in0=f_even, in1=f_even, op=Alu.mult)
        rsum = const.tile([P, 1], f32)
        nc.vector.tensor_reduce(
            out=rsum[:], in_=sq[:], axis=mybir.AxisListType.X, op=Alu.add
        )
        n2_ps = psmall.tile([1, 1], f32, tag="n2")
        nc.tensor.matmul(out=n2_ps[:], lhsT=ones_col[:], rhs=rsum[:], start=True, stop=True)
        inv_sb = const.tile([1, 1], f32)
        nc.vector.reciprocal(out=inv_sb[:], in_=n2_ps[:])
        invb_ps = psmall.tile([P, 1], f32, tag="invb")
        nc.tensor.matmul(out=invb_ps[:], lhsT=ones_row[:], rhs=inv_sb[:], start=True, stop=True)
        invb = const.tile([P, 1], f32)
        nc.vector.tensor_copy(invb[:], invb_ps[:])

        # ---- gate chain (software-pipelined emission: term k's transpose/recurrence
        #      halves interleave with term k+1's matmul rounds so the PE never
        #      queues behind the fm-copy latency)
        w_cur = f_emb  # v_0 of gate 0 (chunk layout, acc-pool tile)
        term_idx = 0
        for g in range(NGATE):
            lam = lams[g]
            c = coeffs[g]
            K = Ks[g]
            beta2 = float((lam / 2.0) ** 2)

            # raw G chunks, DMA'd straight from DRAM (bf16, host-converted)
            gts = []
            for ch in range(NCH):
                gt = gpool.tile([P, DIM], bf16, tag=f"G{ch}", name=f"G{g}_{ch}")
                nc.sync.dma_start(out=gt[:], in_=gens_d.ap()[g, P * ch : P * (ch + 1), :])
                gts.append(gt)

            # v0 in bf16 form for the k=1 matvec lhsT (DVE cast rounds)
            v0r = wpool.tile([P, 2 * NCH], bf16, tag="w", name=f"v0r{g}")
            nc.vector.tensor_copy(v0r[:], w_cur[:])
            acc = apool.tile([P, 2 * NCH], f32, tag="acc", name=f"acc{g}")
            # acc = c0 * v0
            nc.vector.tensor_scalar_mul(acc[:], w_cur[:], float(c[0].real))

            # per-term state: w[k] (bf16 chunk layout), fm/Tt/Y tiles
            w = {0: v0r}
            fm = {}
            Tt = {}
            Y = {}
            gbase = term_idx

            def emit_R(k, half):
                # 16 matmuls: chunks 4*half..4*half+3, all 4 groups col-tiled
                if half == 0:
                    Y[k] = pspool.tile([P, GW], f32, tag="Y", name=f"Y{gbase + k}")
                for ch in range(4 * half, 4 * half + 4):
                    lhs = w[k - 1][:, 2 * ch : 2 * ch + 2]
                    for j in range(NGRP):
                        nc.tensor.matmul(
                            out=Y[k][32 * j : 32 * j + 2, :],
                            lhsT=lhs,
                            rhs=gts[ch][:, GW * j : GW * (j + 1)],
                            start=(ch == 0),
                            stop=(ch == NCH - 1),
                            tile_position=(0, 32 * j),
                        )

            def emit_C(k):
                # PSUM -> SBUF staging; fm2 is emitted inside emit_T(k,0) after
                # the half-0 recurrence so it doesn't delay it on the DVE queue
                fm[k] = fmpool.tile([P, GW], f32, tag="fm", name=f"fm{gbase + k}")
                s05 = 0.5 if k == 1 else 1.0
                if k == 1:
                    nc.vector.tensor_scalar_mul(fm[k][0:2, :], Y[k][0:2, :], s05)
                else:
                    nc.vector.tensor_copy(fm[k][0:2, :], Y[k][0:2, :])
                nc.scalar.activation(out=fm[k][32:34, :], in_=Y[k][32:34, :], func=Copy, scale=s05)
                nc.scalar.activation(out=fm[k][96:98, :], in_=Y[k][96:98, :], func=Copy, scale=s05)

            def emit_rec(k, cols):
                if k == 1:
                    nc.vector.tensor_copy(w[k][:, cols], Tt[k][:, cols])
                else:
                    # v_k = Tt - beta2 * v_{k-2}
                    nc.vector.scalar_tensor_tensor(
                        out=w[k][:, cols], in0=w[k - 2][:, cols],
                        scalar=-beta2, in1=Tt[k][:, cols],
                        op0=Alu.mult, op1=Alu.add,
                    )

            def emit_T(k, half):
                # 4 transposes [2,128]@32j -> [128,2] chunk layout + recurrence half
                if half == 0:
                    Tt[k] = pspool.tile([P, 2 * NCH], f32, tag="T", name=f"T{gbase + k}")
                    w[k] = wpool.tile([P, 2 * NCH], bf16, tag="w", name=f"w{g}_{k}")
                for q in range(4 * half, 4 * half + 4):
                    j, h = q // 2, q % 2
                    nc.tensor.matmul(
                        Tt[k][:, 2 * q : 2 * q + 2],
                        fm[k][32 * j : 32 * j + 2, P * h : P * (h + 1)],
                        idall[32 * j : 32 * j + 2, :],
                        is_transpose=True,
                        start=(q == 0),
                        stop=(q == NCH - 1),
                        tile_position=(32 * j, 0),
                    )
                emit_rec(k, slice(0, 8) if half == 0 else slice(8, 16))
                if half == 0:
                    # deferred group-2 staging copy (DVE, after the h0 recurrence)
                    s05 = 0.5 if k == 1 else 1.0
                    if k == 1:
                        nc.vector.tensor_scalar_mul(fm[k][64:66, :], Y[k][64:66, :], s05)
                    else:
                        nc.vector.tensor_copy(fm[k][64:66, :], Y[k][64:66, :])

            def emit_A(k):
                # acc += chat_k * sigma_k(v_k)
                ck = c[k] * (2.0 / lam) ** k
                if k % 2 == 0:
                    nc.vector.scalar_tensor_tensor(
                        out=acc[:], in0=w[k][:], scalar=float(ck.real),
                        in1=acc[:], op0=Alu.mult, op1=Alu.add,
                    )
                else:
                    wv = w[k][:].rearrange("p (q s) -> p q s", s=2)
                    av = acc[:].rearrange("p (q s) -> p q s", s=2)
                    b = float(ck.imag)
                    nc.vector.scalar_tensor_tensor(
                        out=av[:, :, 0], in0=wv[:, :, 1], scalar=-b,
                        in1=av[:, :, 0], op0=Alu.mult, op1=Alu.add,
                    )
                    nc.vector.scalar_tensor_tensor(
                        out=av[:, :, 1], in0=wv[:, :, 0], scalar=b,
                        in1=av[:, :, 1], op0=Alu.mult, op1=Alu.add,
                    )

            # ---- pipelined driver
            emit_R(1, 0)
            emit_R(1, 1)
            for k in range(1, K + 1):
                emit_C(k)
                emit_T(k, 0)
                if k < K:
                    emit_R(k + 1, 0)
                emit_T(k, 1)
                if k < K:
                    emit_R(k + 1, 1)
                emit_A(k)
                # release dead references
                for d in (fm, Y, Tt):
                    d.pop(k - 1, None)
                w.pop(k - 2, None)
            term_idx += K

            w_cur = acc  # unnormalized psi after gate g

        # ---- output: |psi|^2 / ||f||^2
        sq2 = const.tile([P, 2 * NCH], f32)
        nc.vector.tensor_tensor(out=sq2[:], in0=w_cur[:], in1=w_cur[:], op=Alu.mult)
        sv = sq2[:].rearrange("p (q s) -> p q s", s=2)
        prob = const.tile([P, NCH], f32)
        nc.vector.tensor_tensor(out=prob[:], in0=sv[:, :, 0], in1=sv[:, :, 1], op=Alu.add)
        nc.vector.tensor_scalar_mul(prob[:], prob[:], invb[:])
        nc.sync.dma_start(out=out_d.ap().rearrange("(q r) -> r q", r=P), in_=prob[:])

    nc.compile()
    return nc


# ---------------------------------------------------------------- entry point
_CACHE = {}


def _prep(theta, gens):
    lams = [_lam_max(gens[g].astype(np.float64)) * 1.03 for g in range(NGATE)]
    coeffs, Ks = [], []
    for g in range(NGATE):
        c, K = _cheb_coeffs(float(theta[g, 0]), lams[g])
        coeffs.append(c)
        Ks.append(max(K, 1))
    return lams, coeffs, Ks


def kernel(feature, theta, gens):
    from concourse.bass_utils import run_bass_kernel_spmd

    import ml_dtypes

    feature = np.ascontiguousarray(feature, dtype=np.float32)
    theta = np.ascontiguousarray(theta, dtype=np.float32)
    gens = np.ascontiguousarray(gens, dtype=np.float32)
    gens_bf = np.ascontiguousarray(gens.astype(ml_dtypes.bfloat16))

    lams, coeffs, Ks = _prep(theta, gens)
    key = (theta.tobytes(), tuple(np.round(lams, 9)), tuple(Ks))
    if key not in _CACHE:
        _CACHE[key] = build_program(lams, coeffs, Ks)
    nc = _CACHE[key]

    in_map = {"feature": feature, "gens": gens_bf}
    res = run_bass_kernel_spmd(
        nc,
        [dict(in_map) for _ in range(8)],
        core_ids=list(range(8)),
        trace=False,
    )
    return np.asarray(res.results[0]["out"], dtype=np.float32)


if __name__ == "__main__":
    d = np.load("/root/problem/ref_cache.npz")
    out = kernel(d["feature"], d["theta"], d["gens"])
    exp = d["expected"]
    rel = np.linalg.norm(out - exp) / np.linalg.norm(exp)
    print("l2 rel err:", rel)
    print("max abs err:", np.abs(out - exp).max())
